# revision 11
# baseline (speedup 1.0000x reference)
"""Trainium2 Bass kernel for nn_AgeUGP_v1 (gnn_message_passing, 8 cores).

Math: the reference's per-sample gather + segment-sum + first linear layer
are all linear in snp, so they fold into one dense matmul:

    sample_h = snp @ M            (M sparse [NS, NG], nnz = N_NODES)
    h_pre    = sample_h @ W1.T = snp @ (M @ W1.T) = snp @ W_eff

with W_eff[s, d] = filt_mean[s] * sum_{n: snp_ids[n]=s} W1[d, segment_ids[n]].

The device work is the dense [128, 500000] @ [500000, 64] matmul — the part
that touches all 256 MB of snp. It is K-sharded over the 8 NeuronCores
(contraction sharding: each core gets 62500 rows of snp^T and of W_eff, and
produces a [64, 128] partial of h_pre^T, accumulated in fp32 PSUM from bf16
operands). Partials are summed on host, and the tiny MLP tail + scalar
losses ([128, 64]-sized math, ~0.003% of the FLOPs) run exactly on host.

Host-side prep per call is index/weight preprocessing only:
O(N_NODES * D) scatter-add for W_eff and the snp transpose/cast/shard.
"""

import numpy as np
import ml_dtypes

import concourse.bass as bass
import concourse.bacc as bacc
import concourse.mybir as mybir
import concourse.tile as tile
from concourse.bass_utils import run_bass_kernel_spmd

B = 128
NS = 500_000
NG = 18_000
D = 64
NF = 8
NCORES = 8
P = 128
KPC = NS // NCORES            # 62500 contraction rows per core
KTILES = (KPC + P - 1) // P   # 489
KPAD = KTILES * P             # 62592
import os as _os

CHUNK = int(_os.environ.get("K_CHUNK", "64"))   # k-tiles per DMA chunk
SBUFS = int(_os.environ.get("K_BUFS", "4"))     # tile-pool slots per operand
RAMP = int(_os.environ.get("K_RAMP", "16"))     # ramp-up chunk size (0 = off)
BN_EPS = 1e-5
BF16 = np.dtype(ml_dtypes.bfloat16)

# Set by callers that want profiling; results of the last device run.
TRACE = False
TRACE_CORES = None
LAST_RESULTS = None

_NC_CACHE = None


def _build_nc():
    """One SPMD Bass program: h_pt[d, b] = sum_k w_eff[k, d] * snp_t[k, b].

    DRAM inputs are host-interleaved so every DMA reads a fully contiguous
    span per partition:
      snp_t [P, KTILES*B]: row p, cols [kt*B:(kt+1)*B] = snp^T[kt*128+p, :]
      w_eff [P, KTILES*D]: row p, cols [kt*D:(kt+1)*D] = W_eff[kt*128+p, :]
    so columns [n*B:(n+1)*B] of an SBUF chunk are k-tile n with k on
    partitions, exactly what the matmul wants.
    """
    global _NC_CACHE
    if _NC_CACHE is not None:
        return _NC_CACHE

    nc = bacc.Bacc()
    snp_d = nc.declare_dram_parameter(
        "snp_t", [P, KTILES * B], mybir.dt.bfloat16, isOutput=False
    )
    w_d = nc.declare_dram_parameter(
        "w_eff", [P, KTILES * D], mybir.dt.bfloat16, isOutput=False
    )
    out_d = nc.declare_dram_parameter("h_pt", [D, B], mybir.dt.float32, isOutput=True)

    with tile.TileContext(nc) as tc:
        with (
            tc.tile_pool(name="snp_pool", bufs=SBUFS) as snp_pool,
            tc.tile_pool(name="w_pool", bufs=SBUFS) as w_pool,
            tc.tile_pool(name="psum", bufs=1, space="PSUM") as psum_pool,
            tc.tile_pool(name="out_pool", bufs=1) as out_pool,
        ):
            acc = psum_pool.tile([D, B], mybir.dt.float32)
            # Ramp-up schedule: small first chunks so the HW-DGE descriptor
            # generator (the serial resource at kernel start) gets all 16
            # queues streaming quickly, then steady CHUNK-sized loads.
            sizes = []
            for ramp in (RAMP, RAMP):
                if ramp and KTILES - sum(sizes) > ramp:
                    sizes.append(ramp)
            while sum(sizes) < KTILES:
                sizes.append(min(CHUNK, KTILES - sum(sizes)))
            t0 = 0
            for nt in sizes:
                s_tile = snp_pool.tile([P, CHUNK * B], mybir.dt.bfloat16, tag="s")
                w_tile = w_pool.tile([P, CHUNK * D], mybir.dt.bfloat16, tag="w")
                nc.sync.dma_start(
                    out=s_tile[:, : nt * B], in_=snp_d[:, t0 * B : (t0 + nt) * B]
                )
                nc.sync.dma_start(
                    out=w_tile[:, : nt * D], in_=w_d[:, t0 * D : (t0 + nt) * D]
                )
                for n in range(nt):
                    kt = t0 + n
                    nc.tensor.matmul(
                        acc[:, :],
                        w_tile[:, n * D : (n + 1) * D],   # lhsT [128k, 64d]
                        s_tile[:, n * B : (n + 1) * B],   # rhs  [128k, 128b]
                        start=(kt == 0),
                        stop=(kt == KTILES - 1),
                    )
                t0 += nt
            out_sb = out_pool.tile([D, B], mybir.dt.float32)
            nc.vector.tensor_copy(out_sb[:, :], acc[:, :])
            nc.sync.dma_start(out=out_d[:, :], in_=out_sb[:, :])

    nc.finalize()
    _NC_CACHE = nc
    return nc


def _build_w_eff(filters, W1, snp_ids, segment_ids):
    """W_eff[s, d] = filt_mean[s] * sum_{n: snp_ids[n]=s} W1[d, segment_ids[n]]."""
    filt_mean = filters.mean(axis=0)                      # [NS] f32
    snp_ids = snp_ids.astype(np.int64, copy=False)
    seg = segment_ids.astype(np.int64, copy=False)
    W_eff = np.empty((NS, D), np.float32)
    for d in range(D):
        w = W1[d, seg].astype(np.float64)                 # [NN]
        W_eff[:, d] = np.bincount(snp_ids, weights=w, minlength=NS).astype(np.float32)
    W_eff *= filt_mean[:, None]
    return W_eff


def _interleave_kb(a_bf16, free):
    """[KPC, free] bf16 -> [P, KTILES*free] with row p holding rows
    {kt*128+p} of the (zero-padded) input, contiguously per k-tile."""
    pad = np.zeros((KPAD, free), BF16)
    pad[:KPC] = a_bf16
    return np.ascontiguousarray(
        pad.reshape(KTILES, P, free).transpose(1, 0, 2)
    ).reshape(P, KTILES * free)


def _tail(h_pre, inp):
    """Exact (fp64) replica of the reference MLP tail + heads + losses."""
    f = np.float64
    g1, be1, m1, v1 = (inp[k].astype(f) for k in ("g1", "be1", "m1", "v1"))
    W2, b2, g2, be2, m2, v2 = (
        inp[k].astype(f) for k in ("W2", "b2", "g2", "be2", "m2", "v2")
    )
    Wp, bp, Wa, ba = (inp[k].astype(f) for k in ("Wp", "bp", "Wa", "ba"))
    A1, c1, A2, c2 = (inp[k].astype(f) for k in ("A1", "c1", "A2", "c2"))
    b1 = inp["b1"].astype(f)
    age = inp["age"].astype(f)
    labels = inp["labels"].astype(f)

    h = h_pre.astype(f) + b1
    h = g1 * (h - m1) / np.sqrt(v1 + BN_EPS) + be1
    h = np.maximum(h, 0.0)
    feat = h @ W2.T + b2
    feat = g2 * (feat - m2) / np.sqrt(v2 + BN_EPS) + be2
    feat = np.maximum(feat, 0.0)

    original_logits = feat @ Wp.T + bp                     # [B, 1]
    age_norm = (age - 40.0) / 30.0
    age_pred = 1.0 / (1.0 + np.exp(-(feat @ Wa.T + ba)))   # [B, 1]
    age_loss = np.mean(np.abs(age_pred[:, 0] - age_norm))

    pos_trans = np.maximum(age_norm[:, None] @ A1.T + c1, 0.0) @ A2.T + c2
    e = np.exp(pos_trans - pos_trans.max(axis=1, keepdims=True))
    pos_probs = e / e.sum(axis=1, keepdims=True)           # [B, 2]

    p = np.clip(1.0 / (1.0 + np.exp(-original_logits)), 1e-7, 1.0 - 1e-7)
    # updated_dist = [1-p, p] @ [[1, 0], [pos_probs]] -> col 1 = p * pos_probs[:, 1]
    updated_probs = (p[:, 0] * pos_probs[:, 1])[:, None]
    pos_mask = (labels == 1.0).astype(f)
    per_bce = -(
        p[:, 0] * np.log(updated_probs[:, 0])
        + (1.0 - p[:, 0]) * np.log1p(-updated_probs[:, 0])
    )
    consistency_loss = np.sum(per_bce * pos_mask) / np.sum(pos_mask)
    neg_mask = (1.0 - labels)[:, None]
    final_probs = (1.0 - neg_mask) * p + neg_mask * updated_probs
    final_logits = np.log(final_probs / (1.0 - final_probs + 1e-7))
    final_loss = consistency_loss + 0.5 * age_loss

    return (
        final_logits.astype(np.float32),
        original_logits.astype(np.float32),
        np.float32(final_loss),
    )


def kernel(**inputs):
    global LAST_RESULTS
    inp = {k: np.asarray(v) for k, v in inputs.items()}
    snp = inp["snp"].astype(np.float32, copy=False)        # [B, NS]

    W_eff = _build_w_eff(
        inp["filters"].astype(np.float32, copy=False),
        inp["W1"].astype(np.float32, copy=False),
        inp["snp_ids"],
        inp["segment_ids"],
    )

    snp_bf = snp.astype(BF16)                              # [B, NS]
    W_bf = W_eff.astype(BF16)                              # [NS, D]
    in_maps = []
    for c in range(NCORES):
        sl = slice(c * KPC, (c + 1) * KPC)
        st = _interleave_kb(np.ascontiguousarray(snp_bf[:, sl].T), B)
        wt = _interleave_kb(W_bf[sl], D)
        in_maps.append({"snp_t": st, "w_eff": wt})

    nc = _build_nc()
    res = run_bass_kernel_spmd(
        nc,
        in_maps,
        list(range(NCORES)),
        trace=TRACE,
        trace_cores=TRACE_CORES,
    )
    LAST_RESULTS = res

    h_pt = np.zeros((D, B), np.float64)
    for r in res.results:
        h_pt += r["h_pt"].astype(np.float64)
    h_pre = h_pt.T                                         # [B, D]
    return _tail(h_pre, inp)


# revision 13
# speedup vs baseline: 1.0014x; 1.0014x over previous
"""Trainium2 Bass kernel for nn_AgeUGP_v1 (gnn_message_passing, 8 cores).

Math: the reference's per-sample gather + segment-sum + first linear layer
are all linear in snp, so they fold into one dense matmul:

    sample_h = snp @ M            (M sparse [NS, NG], nnz = N_NODES)
    h_pre    = sample_h @ W1.T = snp @ (M @ W1.T) = snp @ W_eff

with W_eff[s, d] = filt_mean[s] * sum_{n: snp_ids[n]=s} W1[d, segment_ids[n]].

The device work is the dense [128, 500000] @ [500000, 64] matmul — the part
that touches all 256 MB of snp. It is K-sharded over the 8 NeuronCores
(contraction sharding: each core gets 62500 rows of snp^T and of W_eff, and
produces a [64, 128] partial of h_pre^T, accumulated in fp32 PSUM from bf16
operands). Partials are summed on host, and the tiny MLP tail + scalar
losses ([128, 64]-sized math, ~0.003% of the FLOPs) run exactly on host.

Host-side prep per call is index/weight preprocessing only:
O(N_NODES * D) scatter-add for W_eff and the snp transpose/cast/shard.
"""

import numpy as np
import ml_dtypes

import concourse.bass as bass
import concourse.bacc as bacc
import concourse.mybir as mybir
import concourse.tile as tile
from concourse.bass_utils import run_bass_kernel_spmd

B = 128
NS = 500_000
NG = 18_000
D = 64
NF = 8
NCORES = 8
P = 128
KPC = NS // NCORES            # 62500 contraction rows per core
KTILES = (KPC + P - 1) // P   # 489
KPAD = KTILES * P             # 62592
import os as _os

CHUNK = int(_os.environ.get("K_CHUNK", "64"))   # k-tiles per DMA chunk
SBUFS = int(_os.environ.get("K_BUFS", "4"))     # tile-pool slots per operand
RAMP = int(_os.environ.get("K_RAMP", "0"))      # ramp-up chunk size (0 = off)
BN_EPS = 1e-5
BF16 = np.dtype(ml_dtypes.bfloat16)

# Set by callers that want profiling; results of the last device run.
TRACE = False
TRACE_CORES = None
LAST_RESULTS = None

_NC_CACHE = None


def _install_profiling_fallbacks():
    """run_bass_kernel_spmd(trace=True) — which BASS_TRACE=1 also triggers —
    imports antenv.axon_hooks, which this image lacks, and uploads artifacts
    to S3, which may have no creds here. Provide a working NTFF hook shim and
    make the upload non-fatal so tracing degrades instead of crashing."""
    import sys
    import types

    try:
        import antenv.axon_hooks  # noqa: F401
    except ImportError:
        hook = None
        try:
            from trn_agent_boot.trn_boot import _ntff_profile_via_ctypes

            hook = _ntff_profile_via_ctypes("/opt/axon/libaxon_pjrt.so")
        except Exception:
            hook = None
        mod = types.ModuleType("antenv.axon_hooks")
        mod.get_axon_ntff_profile_hook = lambda: hook
        mod.set_axon_ntff_profile_hook = lambda h: None
        sys.modules["antenv.axon_hooks"] = mod
        try:
            import antenv

            antenv.axon_hooks = mod
        except Exception:
            pass

    try:
        import concourse.bass_utils as _bu

        _orig_upload = _bu.upload_artifacts

        def _safe_upload(tmpdir):
            try:
                return _orig_upload(tmpdir)
            except Exception:
                return tmpdir

        _bu.upload_artifacts = _safe_upload
    except Exception:
        pass


try:
    _install_profiling_fallbacks()
except Exception:
    pass


def _build_nc():
    """One SPMD Bass program: h_pt[d, b] = sum_k w_eff[k, d] * snp_t[k, b].

    DRAM inputs are host-interleaved so every DMA reads a fully contiguous
    span per partition:
      snp_t [P, KTILES*B]: row p, cols [kt*B:(kt+1)*B] = snp^T[kt*128+p, :]
      w_eff [P, KTILES*D]: row p, cols [kt*D:(kt+1)*D] = W_eff[kt*128+p, :]
    so columns [n*B:(n+1)*B] of an SBUF chunk are k-tile n with k on
    partitions, exactly what the matmul wants.
    """
    global _NC_CACHE
    if _NC_CACHE is not None:
        return _NC_CACHE

    nc = bacc.Bacc()
    snp_d = nc.declare_dram_parameter(
        "snp_t", [P, KTILES * B], mybir.dt.bfloat16, isOutput=False
    )
    w_d = nc.declare_dram_parameter(
        "w_eff", [P, KTILES * D], mybir.dt.bfloat16, isOutput=False
    )
    out_d = nc.declare_dram_parameter("h_pt", [D, B], mybir.dt.float32, isOutput=True)

    with tile.TileContext(nc) as tc:
        with (
            tc.tile_pool(name="snp_pool", bufs=SBUFS) as snp_pool,
            tc.tile_pool(name="w_pool", bufs=SBUFS) as w_pool,
            tc.tile_pool(name="psum", bufs=1, space="PSUM") as psum_pool,
            tc.tile_pool(name="out_pool", bufs=1) as out_pool,
        ):
            acc = psum_pool.tile([D, B], mybir.dt.float32)
            # Ramp-up schedule: small first chunks so the HW-DGE descriptor
            # generator (the serial resource at kernel start) gets all 16
            # queues streaming quickly, then steady CHUNK-sized loads.
            sizes = []
            for ramp in (RAMP, RAMP):
                if ramp and KTILES - sum(sizes) > ramp:
                    sizes.append(ramp)
            while sum(sizes) < KTILES:
                sizes.append(min(CHUNK, KTILES - sum(sizes)))
            t0 = 0
            for nt in sizes:
                s_tile = snp_pool.tile([P, CHUNK * B], mybir.dt.bfloat16, tag="s")
                w_tile = w_pool.tile([P, CHUNK * D], mybir.dt.bfloat16, tag="w")
                nc.sync.dma_start(
                    out=s_tile[:, : nt * B], in_=snp_d[:, t0 * B : (t0 + nt) * B]
                )
                nc.sync.dma_start(
                    out=w_tile[:, : nt * D], in_=w_d[:, t0 * D : (t0 + nt) * D]
                )
                for n in range(nt):
                    kt = t0 + n
                    nc.tensor.matmul(
                        acc[:, :],
                        w_tile[:, n * D : (n + 1) * D],   # lhsT [128k, 64d]
                        s_tile[:, n * B : (n + 1) * B],   # rhs  [128k, 128b]
                        start=(kt == 0),
                        stop=(kt == KTILES - 1),
                    )
                t0 += nt
            out_sb = out_pool.tile([D, B], mybir.dt.float32)
            nc.vector.tensor_copy(out_sb[:, :], acc[:, :])
            nc.sync.dma_start(out=out_d[:, :], in_=out_sb[:, :])

    nc.finalize()
    _NC_CACHE = nc
    return nc


def _build_w_eff(filters, W1, snp_ids, segment_ids):
    """W_eff[s, d] = filt_mean[s] * sum_{n: snp_ids[n]=s} W1[d, segment_ids[n]]."""
    filt_mean = filters.mean(axis=0)                      # [NS] f32
    snp_ids = snp_ids.astype(np.int64, copy=False)
    seg = segment_ids.astype(np.int64, copy=False)
    W_eff = np.empty((NS, D), np.float32)
    for d in range(D):
        w = W1[d, seg].astype(np.float64)                 # [NN]
        W_eff[:, d] = np.bincount(snp_ids, weights=w, minlength=NS).astype(np.float32)
    W_eff *= filt_mean[:, None]
    return W_eff


def _interleave_kb(a_bf16, free):
    """[KPC, free] bf16 -> [P, KTILES*free] with row p holding rows
    {kt*128+p} of the (zero-padded) input, contiguously per k-tile."""
    pad = np.zeros((KPAD, free), BF16)
    pad[:KPC] = a_bf16
    return np.ascontiguousarray(
        pad.reshape(KTILES, P, free).transpose(1, 0, 2)
    ).reshape(P, KTILES * free)


def _tail(h_pre, inp):
    """Exact (fp64) replica of the reference MLP tail + heads + losses."""
    f = np.float64
    g1, be1, m1, v1 = (inp[k].astype(f) for k in ("g1", "be1", "m1", "v1"))
    W2, b2, g2, be2, m2, v2 = (
        inp[k].astype(f) for k in ("W2", "b2", "g2", "be2", "m2", "v2")
    )
    Wp, bp, Wa, ba = (inp[k].astype(f) for k in ("Wp", "bp", "Wa", "ba"))
    A1, c1, A2, c2 = (inp[k].astype(f) for k in ("A1", "c1", "A2", "c2"))
    b1 = inp["b1"].astype(f)
    age = inp["age"].astype(f)
    labels = inp["labels"].astype(f)

    h = h_pre.astype(f) + b1
    h = g1 * (h - m1) / np.sqrt(v1 + BN_EPS) + be1
    h = np.maximum(h, 0.0)
    feat = h @ W2.T + b2
    feat = g2 * (feat - m2) / np.sqrt(v2 + BN_EPS) + be2
    feat = np.maximum(feat, 0.0)

    original_logits = feat @ Wp.T + bp                     # [B, 1]
    age_norm = (age - 40.0) / 30.0
    age_pred = 1.0 / (1.0 + np.exp(-(feat @ Wa.T + ba)))   # [B, 1]
    age_loss = np.mean(np.abs(age_pred[:, 0] - age_norm))

    pos_trans = np.maximum(age_norm[:, None] @ A1.T + c1, 0.0) @ A2.T + c2
    e = np.exp(pos_trans - pos_trans.max(axis=1, keepdims=True))
    pos_probs = e / e.sum(axis=1, keepdims=True)           # [B, 2]

    p = np.clip(1.0 / (1.0 + np.exp(-original_logits)), 1e-7, 1.0 - 1e-7)
    # updated_dist = [1-p, p] @ [[1, 0], [pos_probs]] -> col 1 = p * pos_probs[:, 1]
    updated_probs = (p[:, 0] * pos_probs[:, 1])[:, None]
    pos_mask = (labels == 1.0).astype(f)
    per_bce = -(
        p[:, 0] * np.log(updated_probs[:, 0])
        + (1.0 - p[:, 0]) * np.log1p(-updated_probs[:, 0])
    )
    consistency_loss = np.sum(per_bce * pos_mask) / np.sum(pos_mask)
    neg_mask = (1.0 - labels)[:, None]
    final_probs = (1.0 - neg_mask) * p + neg_mask * updated_probs
    final_logits = np.log(final_probs / (1.0 - final_probs + 1e-7))
    final_loss = consistency_loss + 0.5 * age_loss

    return (
        final_logits.astype(np.float32),
        original_logits.astype(np.float32),
        np.float32(final_loss),
    )


def kernel(**inputs):
    global LAST_RESULTS
    inp = {k: np.asarray(v) for k, v in inputs.items()}
    snp = inp["snp"].astype(np.float32, copy=False)        # [B, NS]

    W_eff = _build_w_eff(
        inp["filters"].astype(np.float32, copy=False),
        inp["W1"].astype(np.float32, copy=False),
        inp["snp_ids"],
        inp["segment_ids"],
    )

    snp_bf = snp.astype(BF16)                              # [B, NS]
    W_bf = W_eff.astype(BF16)                              # [NS, D]
    in_maps = []
    for c in range(NCORES):
        sl = slice(c * KPC, (c + 1) * KPC)
        st = _interleave_kb(np.ascontiguousarray(snp_bf[:, sl].T), B)
        wt = _interleave_kb(W_bf[sl], D)
        in_maps.append({"snp_t": st, "w_eff": wt})

    nc = _build_nc()
    res = run_bass_kernel_spmd(
        nc,
        in_maps,
        list(range(NCORES)),
        trace=TRACE,
        trace_cores=TRACE_CORES,
    )
    LAST_RESULTS = res

    h_pt = np.zeros((D, B), np.float64)
    for r in res.results:
        h_pt += r["h_pt"].astype(np.float64)
    h_pre = h_pt.T                                         # [B, D]
    return _tail(h_pre, inp)


# revision 17
# speedup vs baseline: 1.1119x; 1.1103x over previous
"""Trainium2 Bass kernel for nn_AgeUGP_v1 (gnn_message_passing, 8 cores).

Math: the reference's per-sample gather + segment-sum + first linear layer
are all linear in snp, so they fold into one dense matmul:

    sample_h = snp @ M            (M sparse [NS, NG], nnz = N_NODES)
    h_pre    = sample_h @ W1.T = snp @ (M @ W1.T) = snp @ W_eff

with W_eff[s, d] = filt_mean[s] * sum_{n: snp_ids[n]=s} W1[d, segment_ids[n]].

The device work is the dense [128, 500000] @ [500000, 64] matmul — the part
that touches all 256 MB of snp. It is K-sharded over the 8 NeuronCores
(contraction sharding: each core gets 62500 rows of snp^T and of W_eff, and
produces a [64, 128] partial of h_pre^T, accumulated in fp32 PSUM from bf16
operands). Partials are summed on host, and the tiny MLP tail + scalar
losses ([128, 64]-sized math, ~0.003% of the FLOPs) run exactly on host.

Host-side prep per call is index/weight preprocessing only:
O(N_NODES * D) scatter-add for W_eff and the snp transpose/cast/shard.
"""

import numpy as np
import ml_dtypes

import concourse.bass as bass
import concourse.bacc as bacc
import concourse.mybir as mybir
import concourse.tile as tile
from concourse.bass_utils import run_bass_kernel_spmd

B = 128
NS = 500_000
NG = 18_000
D = 64
NF = 8
NCORES = 8
P = 128
KPC = NS // NCORES            # 62500 contraction rows per core
KTILES = (KPC + P - 1) // P   # 489
KPAD = KTILES * P             # 62592
import os as _os

CHUNK = int(_os.environ.get("K_CHUNK", "64"))   # k-tiles per DMA chunk
SBUFS = int(_os.environ.get("K_BUFS", "4"))     # tile-pool slots per operand
RAMP = int(_os.environ.get("K_RAMP", "0"))      # ramp-up chunk size (0 = off)
BN_EPS = 1e-5
BF16 = np.dtype(ml_dtypes.bfloat16)

# Set by callers that want profiling; results of the last device run.
TRACE = False
TRACE_CORES = None
LAST_RESULTS = None

_NC_CACHE = None


def _install_profiling_fallbacks():
    """run_bass_kernel_spmd(trace=True) — which BASS_TRACE=1 also triggers —
    imports antenv.axon_hooks, which this image lacks, and uploads artifacts
    to S3, which may have no creds here. Provide a working NTFF hook shim and
    make the upload non-fatal so tracing degrades instead of crashing."""
    import sys
    import types

    try:
        import antenv.axon_hooks  # noqa: F401
    except ImportError:
        hook = None
        try:
            from trn_agent_boot.trn_boot import _ntff_profile_via_ctypes

            hook = _ntff_profile_via_ctypes("/opt/axon/libaxon_pjrt.so")
        except Exception:
            hook = None
        mod = types.ModuleType("antenv.axon_hooks")
        mod.get_axon_ntff_profile_hook = lambda: hook
        mod.set_axon_ntff_profile_hook = lambda h: None
        sys.modules["antenv.axon_hooks"] = mod
        try:
            import antenv

            antenv.axon_hooks = mod
        except Exception:
            pass

    try:
        import concourse.bass_utils as _bu

        _orig_upload = _bu.upload_artifacts

        def _safe_upload(tmpdir):
            try:
                return _orig_upload(tmpdir)
            except Exception:
                return tmpdir

        _bu.upload_artifacts = _safe_upload
    except Exception:
        pass


try:
    _install_profiling_fallbacks()
except Exception:
    pass


def _build_nc():
    """One SPMD Bass program: h_pt[d, b] = sum_k w_eff[k, d] * snp_t[k, b].

    DRAM inputs are host-interleaved so every DMA reads a fully contiguous
    span per partition:
      snp_t [P, KTILES*B]: row p, cols [kt*B:(kt+1)*B] = snp^T[kt*128+p, :]
      w_eff [P, KTILES*D]: row p, cols [kt*D:(kt+1)*D] = W_eff[kt*128+p, :]
    so columns [n*B:(n+1)*B] of an SBUF chunk are k-tile n with k on
    partitions, exactly what the matmul wants.
    """
    global _NC_CACHE
    if _NC_CACHE is not None:
        return _NC_CACHE

    nc = bacc.Bacc()
    snp_d = nc.declare_dram_parameter(
        "snp_t", [P, KTILES * B], mybir.dt.bfloat16, isOutput=False
    )
    w_d = nc.declare_dram_parameter(
        "w_eff", [P, KTILES * D], mybir.dt.bfloat16, isOutput=False
    )
    out_d = nc.declare_dram_parameter("h_pt", [D, B], mybir.dt.float32, isOutput=True)

    with tile.TileContext(nc) as tc:
        with (
            tc.tile_pool(name="snp_pool", bufs=SBUFS) as snp_pool,
            tc.tile_pool(name="w_pool", bufs=SBUFS) as w_pool,
            tc.tile_pool(name="psum", bufs=1, space="PSUM") as psum_pool,
            tc.tile_pool(name="out_pool", bufs=1) as out_pool,
        ):
            acc = psum_pool.tile([D, B], mybir.dt.float32)
            # Ramp-up schedule: small first chunks so the HW-DGE descriptor
            # generator (the serial resource at kernel start) gets all 16
            # queues streaming quickly, then steady CHUNK-sized loads.
            sizes = []
            for ramp in (RAMP, RAMP):
                if ramp and KTILES - sum(sizes) > ramp:
                    sizes.append(ramp)
            while sum(sizes) < KTILES:
                sizes.append(min(CHUNK, KTILES - sum(sizes)))
            t0 = 0
            for nt in sizes:
                s_tile = snp_pool.tile([P, CHUNK * B], mybir.dt.bfloat16, tag="s")
                w_tile = w_pool.tile([P, CHUNK * D], mybir.dt.bfloat16, tag="w")
                nc.sync.dma_start(
                    out=s_tile[:, : nt * B], in_=snp_d[:, t0 * B : (t0 + nt) * B]
                )
                nc.sync.dma_start(
                    out=w_tile[:, : nt * D], in_=w_d[:, t0 * D : (t0 + nt) * D]
                )
                for n in range(nt):
                    kt = t0 + n
                    nc.tensor.matmul(
                        acc[:, :],
                        w_tile[:, n * D : (n + 1) * D],   # lhsT [128k, 64d]
                        s_tile[:, n * B : (n + 1) * B],   # rhs  [128k, 128b]
                        start=(kt == 0),
                        stop=(kt == KTILES - 1),
                    )
                t0 += nt
            out_sb = out_pool.tile([D, B], mybir.dt.float32)
            nc.vector.tensor_copy(out_sb[:, :], acc[:, :])
            nc.sync.dma_start(out=out_d[:, :], in_=out_sb[:, :])

    nc.finalize()
    _NC_CACHE = nc
    return nc


def _build_nc_wpre():
    """Variant: W_eff fully resident in SBUF (loaded as NPIECE big DMAs up
    front), snp streamed in NPIECE big chunks. Halves the HW-DGE descriptor
    count vs the chunked build (fewer, larger per-partition runs)."""
    global _NC_CACHE
    if _NC_CACHE is not None:
        return _NC_CACHE

    NPIECE = 4
    nchunks = (KTILES + CHUNK - 1) // CHUNK
    cpp = (nchunks + NPIECE - 1) // NPIECE          # chunks per W piece
    piece_start = [min(i * cpp * CHUNK, KTILES) for i in range(NPIECE + 1)]
    sizes = [piece_start[i + 1] - piece_start[i] for i in range(NPIECE)]

    nc = bacc.Bacc()
    snp_d = nc.declare_dram_parameter(
        "snp_t", [P, KTILES * B], mybir.dt.bfloat16, isOutput=False
    )
    w_d = nc.declare_dram_parameter(
        "w_eff", [P, KTILES * D], mybir.dt.bfloat16, isOutput=False
    )
    out_d = nc.declare_dram_parameter("h_pt", [D, B], mybir.dt.float32, isOutput=True)

    with tile.TileContext(nc) as tc:
        with (
            tc.tile_pool(name="w_pool", bufs=1) as w_pool,
            tc.tile_pool(name="snp_pool", bufs=SBUFS) as snp_pool,
            tc.tile_pool(name="psum", bufs=1, space="PSUM") as psum_pool,
            tc.tile_pool(name="out_pool", bufs=1) as out_pool,
        ):
            acc = psum_pool.tile([D, B], mybir.dt.float32)
            w_tiles = []
            t0 = 0
            for i, nt in enumerate(sizes):
                w_t = w_pool.tile([P, nt * D], mybir.dt.bfloat16, tag=f"w{i}")
                nc.sync.dma_start(
                    out=w_t[:, :], in_=w_d[:, t0 * D : (t0 + nt) * D]
                )
                w_tiles.append(w_t)
                t0 += nt
            for c in range(nchunks):
                t0 = c * CHUNK
                nt = min(CHUNK, KTILES - t0)
                piece = c // cpp
                poff = t0 - piece_start[piece]      # k-tile offset in W piece
                s_tile = snp_pool.tile([P, CHUNK * B], mybir.dt.bfloat16, tag="s")
                nc.sync.dma_start(
                    out=s_tile[:, : nt * B], in_=snp_d[:, t0 * B : (t0 + nt) * B]
                )
                for n in range(nt):
                    kt = t0 + n
                    nc.tensor.matmul(
                        acc[:, :],
                        w_tiles[piece][:, (poff + n) * D : (poff + n + 1) * D],
                        s_tile[:, n * B : (n + 1) * B],
                        start=(kt == 0),
                        stop=(kt == KTILES - 1),
                    )
            out_sb = out_pool.tile([D, B], mybir.dt.float32)
            nc.vector.tensor_copy(out_sb[:, :], acc[:, :])
            nc.sync.dma_start(out=out_d[:, :], in_=out_sb[:, :])

    nc.finalize()
    _NC_CACHE = nc
    return nc


if _os.environ.get("K_WPRE") == "1":
    _build_nc = _build_nc_wpre  # noqa: F811


def _build_w_eff(filters, W1, snp_ids, segment_ids):
    """W_eff[s, d] = filt_mean[s] * sum_{n: snp_ids[n]=s} W1[d, segment_ids[n]]."""
    filt_mean = filters.mean(axis=0)                      # [NS] f32
    snp_ids = snp_ids.astype(np.int64, copy=False)
    seg = segment_ids.astype(np.int64, copy=False)
    W_eff = np.empty((NS, D), np.float32)
    for d in range(D):
        w = W1[d, seg].astype(np.float64)                 # [NN]
        W_eff[:, d] = np.bincount(snp_ids, weights=w, minlength=NS).astype(np.float32)
    W_eff *= filt_mean[:, None]
    return W_eff


def _interleave_kb(a_bf16, free):
    """[KPC, free] bf16 -> [P, KTILES*free] with row p holding rows
    {kt*128+p} of the (zero-padded) input, contiguously per k-tile."""
    pad = np.zeros((KPAD, free), BF16)
    pad[:KPC] = a_bf16
    return np.ascontiguousarray(
        pad.reshape(KTILES, P, free).transpose(1, 0, 2)
    ).reshape(P, KTILES * free)


def _tail(h_pre, inp):
    """Exact (fp64) replica of the reference MLP tail + heads + losses."""
    f = np.float64
    g1, be1, m1, v1 = (inp[k].astype(f) for k in ("g1", "be1", "m1", "v1"))
    W2, b2, g2, be2, m2, v2 = (
        inp[k].astype(f) for k in ("W2", "b2", "g2", "be2", "m2", "v2")
    )
    Wp, bp, Wa, ba = (inp[k].astype(f) for k in ("Wp", "bp", "Wa", "ba"))
    A1, c1, A2, c2 = (inp[k].astype(f) for k in ("A1", "c1", "A2", "c2"))
    b1 = inp["b1"].astype(f)
    age = inp["age"].astype(f)
    labels = inp["labels"].astype(f)

    h = h_pre.astype(f) + b1
    h = g1 * (h - m1) / np.sqrt(v1 + BN_EPS) + be1
    h = np.maximum(h, 0.0)
    feat = h @ W2.T + b2
    feat = g2 * (feat - m2) / np.sqrt(v2 + BN_EPS) + be2
    feat = np.maximum(feat, 0.0)

    original_logits = feat @ Wp.T + bp                     # [B, 1]
    age_norm = (age - 40.0) / 30.0
    age_pred = 1.0 / (1.0 + np.exp(-(feat @ Wa.T + ba)))   # [B, 1]
    age_loss = np.mean(np.abs(age_pred[:, 0] - age_norm))

    pos_trans = np.maximum(age_norm[:, None] @ A1.T + c1, 0.0) @ A2.T + c2
    e = np.exp(pos_trans - pos_trans.max(axis=1, keepdims=True))
    pos_probs = e / e.sum(axis=1, keepdims=True)           # [B, 2]

    p = np.clip(1.0 / (1.0 + np.exp(-original_logits)), 1e-7, 1.0 - 1e-7)
    # updated_dist = [1-p, p] @ [[1, 0], [pos_probs]] -> col 1 = p * pos_probs[:, 1]
    updated_probs = (p[:, 0] * pos_probs[:, 1])[:, None]
    pos_mask = (labels == 1.0).astype(f)
    per_bce = -(
        p[:, 0] * np.log(updated_probs[:, 0])
        + (1.0 - p[:, 0]) * np.log1p(-updated_probs[:, 0])
    )
    consistency_loss = np.sum(per_bce * pos_mask) / np.sum(pos_mask)
    neg_mask = (1.0 - labels)[:, None]
    final_probs = (1.0 - neg_mask) * p + neg_mask * updated_probs
    final_logits = np.log(final_probs / (1.0 - final_probs + 1e-7))
    final_loss = consistency_loss + 0.5 * age_loss

    return (
        final_logits.astype(np.float32),
        original_logits.astype(np.float32),
        np.float32(final_loss),
    )


def kernel(**inputs):
    global LAST_RESULTS
    inp = {k: np.asarray(v) for k, v in inputs.items()}
    snp = inp["snp"].astype(np.float32, copy=False)        # [B, NS]

    W_eff = _build_w_eff(
        inp["filters"].astype(np.float32, copy=False),
        inp["W1"].astype(np.float32, copy=False),
        inp["snp_ids"],
        inp["segment_ids"],
    )

    snp_bf = snp.astype(BF16)                              # [B, NS]
    W_bf = W_eff.astype(BF16)                              # [NS, D]
    in_maps = []
    for c in range(NCORES):
        sl = slice(c * KPC, (c + 1) * KPC)
        st = _interleave_kb(np.ascontiguousarray(snp_bf[:, sl].T), B)
        wt = _interleave_kb(W_bf[sl], D)
        in_maps.append({"snp_t": st, "w_eff": wt})

    nc = _build_nc()
    res = run_bass_kernel_spmd(
        nc,
        in_maps,
        list(range(NCORES)),
        trace=TRACE,
        trace_cores=TRACE_CORES,
    )
    LAST_RESULTS = res

    h_pt = np.zeros((D, B), np.float64)
    for r in res.results:
        h_pt += r["h_pt"].astype(np.float64)
    h_pre = h_pt.T                                         # [B, D]
    return _tail(h_pre, inp)
